# revision 8
# baseline (speedup 1.0000x reference)
"""Trainium2 Bass kernel for nn_NeuralSplineBase (histogram_binning).

y(x) = sum_j w_j * relu(x - t_j)^3 + b0*x^2 + c1*x + c0   per (b, e, c),
with w_j the jumps of the piecewise cubic's leading coefficients
(C2 truncated-power identity).  Pure data parallel over B on 8 cores.

Per core (1 batch element): CN images [128, FREE].
Per third [128, THIRD]: relu-shifts (DVE tensor_scalar), squares (ACT),
cubes (DVE scalar_tensor_tensor).  Per 512-slice: 3 PSUM accumulators
(one per e) fed by 11 sources x 4 quadrant 32x32 scaled-identity
matmuls at rotated tile positions sigma_e(q) = (q+e)%4 so the three
e-streams use disjoint PE sub-arrays.  Evict = ACT Identity(+c0 bias);
rotation undone by split output DMAs.
"""

import numpy as np

NKNOT = 10
STEP = 1.0 / (NKNOT - 1.0)
NVALS = 255
B, E, C = 8, 3, 3
H = W = 768
HW = H * W            # 589824
P = 128
FREE = HW // P        # 4608
NTH = 3               # thirds per image
THIRD = FREE // NTH   # 1536
NSRC = 11             # 9 cubes + x^2 + x

FULL_GEOM = dict(CN=C, NTH=NTH, THIRD=THIRD, rotate=True)


# ---------------------------------------------------------------- host math
def _spline_matrix(n=NKNOT):
    step = 1.0 / (n - 1.0)
    mat = 4.0 * np.eye(n - 2)
    np.fill_diagonal(mat[1:, :-1], 1.0)
    np.fill_diagonal(mat[:-1, 1:], 1.0)
    A = 6.0 * np.linalg.inv(mat) / step**2
    z = np.zeros(n - 2)
    A = np.vstack([z, A, z])
    Bm = np.zeros([n - 2, n])
    np.fill_diagonal(Bm, 1.0)
    np.fill_diagonal(Bm[:, 1:], -2.0)
    np.fill_diagonal(Bm[:, 2:], 1.0)
    return (A @ Bm).astype(np.float32)


_MAT64 = _spline_matrix().astype(np.float64)


def _piece_coeffs(ys_total):
    """ys_total [..., 10] -> a,b,c,d [..., 9] float64."""
    y = ys_total.astype(np.float64)
    h = float(STEP)
    M = np.einsum("ij,...j->...i", _MAT64, y)
    a = (M[..., 1:] - M[..., :-1]) / (6.0 * h)
    b = M[..., :-1] / 2.0
    c = (y[..., 1:] - y[..., :-1]) / h - (M[..., 1:] + 2.0 * M[..., :-1]) * (h / 6.0)
    d = y[..., :-1]
    return a, b, c, d


def _weights(ys_total):
    """[..., 10] -> (wcube [..., 9], wx2, wx, w1) float64."""
    a, b, c, d = _piece_coeffs(ys_total)
    wc = np.empty(a.shape)
    wc[..., 0] = a[..., 0]
    wc[..., 1:] = a[..., 1:] - a[..., :-1]
    return wc, b[..., 0], c[..., 0], d[..., 0]


def host_inputs(batch_b, ys_b, geom):
    """Per-core input arrays.  batch_b [CN, P, FREEn] f32 view,
    ys_b [E, C, 10] f32 (full ys row for this batch element)."""
    CN = geom["CN"]
    nblk = CN * E * NSRC
    identity = np.arange(NKNOT, dtype=np.float32) / np.float32(NKNOT - 1.0)
    wc, wx2, wx, w1 = _weights(ys_b + identity)  # [E, C, ...] f64

    vals = np.arange(NVALS, dtype=np.float64) / 255.0
    vfeat = np.zeros((12, 256), np.float32)
    for j in range(9):
        vfeat[j, :NVALS] = (np.maximum(vals - j * STEP, 0.0) ** 3).astype(np.float32)
    vfeat[9, :NVALS] = (vals**2).astype(np.float32)
    vfeat[10, :NVALS] = vals.astype(np.float32)
    vfeat[11, :NVALS] = 1.0

    eye32 = np.eye(32, dtype=np.float32)
    statv = np.zeros((P, nblk * 32), np.float32)
    coefv = np.zeros((P, CN * E), np.float32)
    spcoefv = np.zeros((12, E * C), np.float32)
    for c in range(CN):
        for e in range(E):
            ws = np.concatenate(
                [wc[e, c], [wx2[e, c]], [wx[e, c]]]
            ).astype(np.float32)
            for srci in range(NSRC):
                nb = (c * E + e) * NSRC + srci
                statv[:, nb * 32:(nb + 1) * 32] = np.tile(eye32 * ws[srci], (4, 1))
            coefv[:, c * E + e] = np.float32(w1[e, c])
    for c in range(C):
        for e in range(E):
            r = e * C + c
            spcoefv[0:9, r] = wc[e, c].astype(np.float32)
            spcoefv[9, r] = np.float32(wx2[e, c])
            spcoefv[10, r] = np.float32(wx[e, c])
            spcoefv[11, r] = np.float32(w1[e, c])
    return {
        "xb": np.ascontiguousarray(batch_b.astype(np.float32)),
        "stat": statv,
        "coef": coefv,
        "spfeat": vfeat,
        "spcoef": spcoefv,
    }


# ------------------------------------------------------------- bass program
def build_program(geom):
    import concourse.bacc as bacc
    import concourse.mybir as mybir
    import concourse.tile as tile

    CN, NTHn, THIRDn = geom["CN"], geom["NTH"], geom["THIRD"]
    rotate = geom["rotate"]
    FREEn = NTHn * THIRDn
    NSLn = THIRDn // 512
    nblk = CN * E * NSRC

    f32 = mybir.dt.float32
    bf16 = mybir.dt.bfloat16
    Alu = mybir.AluOpType
    Act = mybir.ActivationFunctionType

    nc = bacc.Bacc("TRN2", target_bir_lowering=False, debug=False, num_devices=B)
    xb = nc.dram_tensor("xb", [CN, P, FREEn], f32, kind="ExternalInput")
    stat = nc.dram_tensor("stat", [P, nblk * 32], f32, kind="ExternalInput")
    coef = nc.dram_tensor("coef", [P, CN * E], f32, kind="ExternalInput")
    spfeat = nc.dram_tensor("spfeat", [12, 256], f32, kind="ExternalInput")
    spcoef = nc.dram_tensor("spcoef", [12, E * C], f32, kind="ExternalInput")
    out = nc.dram_tensor("out", [E, CN, P, FREEn], f32, kind="ExternalOutput")
    spl = nc.dram_tensor("spl", [E * C, 256], f32, kind="ExternalOutput")

    with tile.TileContext(nc) as tc:
        with (
            tc.tile_pool(name="const", bufs=1) as constp,
            tc.tile_pool(name="xp", bufs=2) as xp,
            tc.tile_pool(name="feat", bufs=1) as featp,
            tc.tile_pool(name="yp", bufs=6) as yp,
            tc.tile_pool(name="psum", bufs=1, space="PSUM") as psp,
            tc.tile_pool(name="psum2", bufs=1, space="PSUM") as psp2,
        ):
            statt = constp.tile([P, nblk * 32], f32)
            nc.sync.dma_start(statt[:], stat.ap())
            # bf16 zero operands for the bank-opening zero-matmul
            zrow = constp.tile([1, 128], bf16)
            nc.vector.memset(zrow[:], 0.0)
            zin = constp.tile([1, 512], bf16)
            nc.vector.memset(zin[:], 0.0)
            coeft = constp.tile([P, CN * E], f32)
            nc.sync.dma_start(coeft[:], coef.ap())

            # ---- splines (tiny): one matmul [12,9]^T @ [12,256]
            spf = constp.tile([12, 256], f32)
            nc.sync.dma_start(spf[:], spfeat.ap())
            spc = constp.tile([12, E * C], f32)
            nc.sync.dma_start(spc[:], spcoef.ap())
            spps = psp2.tile([E * C, 256], f32)
            nc.tensor.matmul(spps[:], spc[:], spf[:], start=True, stop=True)
            spev = constp.tile([E * C, 256], f32)
            nc.vector.tensor_copy(spev[:], spps[:])
            nc.sync.dma_start(spl.ap(), spev[:])

            for c in range(CN):
                for T in range(NTHn):
                    xt = xp.tile([P, THIRDn], f32, tag="x", name=f"x_{c}_{T}")
                    nc.sync.dma_start(
                        xt[:], xb.ap()[c, :, T * THIRDn:(T + 1) * THIRDn]
                    )
                    rsup = featp.tile([P, 9 * THIRDn], f32, tag="r", name=f"r_{c}_{T}")
                    ssup = featp.tile([P, 9 * THIRDn], f32, tag="s", name=f"s_{c}_{T}")
                    for m in range(9):
                        sl = slice(m * THIRDn, (m + 1) * THIRDn)
                        # r_m = relu(x - t_m)   (m=0 -> copy of x)
                        nc.vector.tensor_scalar(
                            rsup[:, sl], xt[:], -m * STEP, 0.0, Alu.add, Alu.max
                        )
                        # s_m = r_m^2
                        nc.scalar.activation(ssup[:, sl], rsup[:, sl], Act.Square)
                        # C_m = (x - t_m) * s_m   (overwrite r slot)
                        nc.vector.scalar_tensor_tensor(
                            rsup[:, sl], xt[:], -m * STEP, ssup[:, sl],
                            Alu.add, Alu.mult,
                        )
                    for s in range(NSLn):
                        ys_t = [
                            psp.tile([P, 512], f32, tag=f"Y{e}",
                                     name=f"Y{e}_{c}_{T}_{s}")
                            for e in range(E)
                        ]
                        for e in range(E):
                            # zero the bank + set has_written everywhere so
                            # the accumulating matmuls below can run in any
                            # order (maximal tile-position concurrency)
                            nc.tensor.matmul(
                                ys_t[e][:, :], zrow[:], zin[:],
                                start=True, stop=False, skip_group_check=True,
                            )
                        for src in range(NSRC):
                            if src < 9:
                                srcap = rsup[:, src * THIRDn + s * 512:
                                             src * THIRDn + s * 512 + 512]
                            elif src == 9:  # x^2
                                srcap = ssup[:, s * 512:s * 512 + 512]
                            else:           # x
                                srcap = xt[:, s * 512:s * 512 + 512]
                            for e in range(E):
                                nb = (c * E + e) * NSRC + src
                                lw = statt[:, nb * 32:(nb + 1) * 32]
                                for q in range(4):
                                    oq = (q + e) % 4 if rotate else q
                                    nc.tensor.matmul(
                                        ys_t[e][32 * oq:32 * oq + 32, :],
                                        lw[32 * q:32 * q + 32, :],
                                        srcap[32 * q:32 * q + 32, :],
                                        start=False,
                                        stop=(src == NSRC - 1),
                                        skip_group_check=True,
                                        tile_position=(32 * q, 32 * oq),
                                    )
                        for e in range(E):
                            yt = yp.tile([P, 512], f32, tag="y",
                                         name=f"yt{e}_{c}_{T}_{s}")
                            nc.scalar.activation(
                                yt[:], ys_t[e][:], Act.Identity,
                                bias=coeft[:, c * E + e:c * E + e + 1],
                            )
                            # undo quadrant rotation: sbuf row 32*((q+e)%4)+p
                            # holds pixel 32*q+p
                            col = slice(T * THIRDn + s * 512,
                                        T * THIRDn + s * 512 + 512)
                            k = 32 * e if rotate else 0
                            if k == 0:
                                nc.sync.dma_start(out.ap()[e, c, :, col], yt[:])
                            else:
                                nc.sync.dma_start(
                                    out.ap()[e, c, 0:P - k, col], yt[k:P, :]
                                )
                                nc.sync.dma_start(
                                    out.ap()[e, c, P - k:P, col], yt[0:k, :]
                                )
    nc.compile()
    return nc


_PROG = None


def _get_prog():
    global _PROG
    if _PROG is None:
        _PROG = build_program(FULL_GEOM)
    return _PROG


# ------------------------------------------------------------------- driver
def kernel(batch, ys):
    """batch [8,3,768,768] f32, ys [8,3,3,10] f32 ->
    (out [8,3,3,768,768] f32, splines [8,3,3,255] f32)."""
    from concourse.bass_utils import run_bass_kernel_spmd

    batch = np.asarray(batch)
    ys = np.asarray(ys)
    in_maps = [
        host_inputs(batch[b].reshape(C, P, FREE), ys[b], FULL_GEOM)
        for b in range(B)
    ]
    nc = _get_prog()
    res = run_bass_kernel_spmd(nc, in_maps, list(range(B)))

    out = np.empty((B, E, C, H, W), np.float32)
    spl = np.empty((B, E, C, NVALS), np.float32)
    for b in range(B):
        out[b] = res.results[b]["out"].reshape(E, C, H, W)
        spl[b] = res.results[b]["spl"][:, :NVALS].reshape(E, C, NVALS)
    return out, spl


# revision 12
# speedup vs baseline: 33380.3950x; 33380.3950x over previous
"""Trainium2 Bass kernel for nn_NeuralSplineBase (histogram_binning).

y(x) = sum_j w_j * relu(x - t_j)^3 + b0*x^2 + c1*x + c0   per (b, e, c),
with w_j the jumps of the piecewise cubic's leading coefficients
(C2 truncated-power identity).  Pure data parallel over B on 8 cores.

Per core (1 batch element): CN images [128, FREE].
Per third [128, THIRD]: relu-shifts (DVE tensor_scalar), squares (ACT),
cubes (DVE scalar_tensor_tensor).  Per 512-slice: 3 PSUM accumulators
(one per e) fed by 11 sources x 4 quadrant 32x32 scaled-identity
matmuls at rotated tile positions sigma_e(q) = (q+e)%4 so the three
e-streams use disjoint PE sub-arrays.  Evict = ACT Identity(+c0 bias);
rotation undone by split output DMAs.
"""

import numpy as np

NKNOT = 10
STEP = 1.0 / (NKNOT - 1.0)
NVALS = 255
B, E, C = 8, 3, 3
H = W = 768
HW = H * W            # 589824
P = 128
FREE = HW // P        # 4608
NTH = 3               # thirds per image
THIRD = FREE // NTH   # 1536
NSRC = 11             # 9 cubes + x^2 + x

FULL_GEOM = dict(CN=C, NTH=NTH, THIRD=THIRD, rotate=True)


# ---------------------------------------------------------------- host math
def _spline_matrix(n=NKNOT):
    step = 1.0 / (n - 1.0)
    mat = 4.0 * np.eye(n - 2)
    np.fill_diagonal(mat[1:, :-1], 1.0)
    np.fill_diagonal(mat[:-1, 1:], 1.0)
    A = 6.0 * np.linalg.inv(mat) / step**2
    z = np.zeros(n - 2)
    A = np.vstack([z, A, z])
    Bm = np.zeros([n - 2, n])
    np.fill_diagonal(Bm, 1.0)
    np.fill_diagonal(Bm[:, 1:], -2.0)
    np.fill_diagonal(Bm[:, 2:], 1.0)
    return (A @ Bm).astype(np.float32)


_MAT64 = _spline_matrix().astype(np.float64)


def _piece_coeffs(ys_total):
    """ys_total [..., 10] -> a,b,c,d [..., 9] float64."""
    y = ys_total.astype(np.float64)
    h = float(STEP)
    M = np.einsum("ij,...j->...i", _MAT64, y)
    a = (M[..., 1:] - M[..., :-1]) / (6.0 * h)
    b = M[..., :-1] / 2.0
    c = (y[..., 1:] - y[..., :-1]) / h - (M[..., 1:] + 2.0 * M[..., :-1]) * (h / 6.0)
    d = y[..., :-1]
    return a, b, c, d


def _weights(ys_total):
    """[..., 10] -> (wcube [..., 9], wx2, wx, w1) float64."""
    a, b, c, d = _piece_coeffs(ys_total)
    wc = np.empty(a.shape)
    wc[..., 0] = a[..., 0]
    wc[..., 1:] = a[..., 1:] - a[..., :-1]
    return wc, b[..., 0], c[..., 0], d[..., 0]


def host_inputs(batch_b, ys_b, geom):
    """Per-core input arrays.  batch_b [CN, P, FREEn] f32 view,
    ys_b [E, C, 10] f32 (full ys row for this batch element)."""
    CN = geom["CN"]
    nblk = CN * E * NSRC
    identity = np.arange(NKNOT, dtype=np.float32) / np.float32(NKNOT - 1.0)
    wc, wx2, wx, w1 = _weights(ys_b + identity)  # [E, C, ...] f64

    vals = np.arange(NVALS, dtype=np.float64) / 255.0
    vfeat = np.zeros((12, 256), np.float32)
    for j in range(9):
        vfeat[j, :NVALS] = (np.maximum(vals - j * STEP, 0.0) ** 3).astype(np.float32)
    vfeat[9, :NVALS] = (vals**2).astype(np.float32)
    vfeat[10, :NVALS] = vals.astype(np.float32)
    vfeat[11, :NVALS] = 1.0

    eye32 = np.eye(32, dtype=np.float32)
    statv = np.zeros((P, nblk * 32), np.float32)
    coefv = np.zeros((P, CN * E), np.float32)
    spcoefv = np.zeros((12, E * C), np.float32)
    for c in range(CN):
        for e in range(E):
            ws = np.concatenate(
                [wc[e, c], [wx2[e, c]], [wx[e, c]]]
            ).astype(np.float32)
            for srci in range(NSRC):
                nb = (c * E + e) * NSRC + srci
                statv[:, nb * 32:(nb + 1) * 32] = np.tile(eye32 * ws[srci], (4, 1))
            coefv[:, c * E + e] = np.float32(w1[e, c])
    for c in range(C):
        for e in range(E):
            r = e * C + c
            spcoefv[0:9, r] = wc[e, c].astype(np.float32)
            spcoefv[9, r] = np.float32(wx2[e, c])
            spcoefv[10, r] = np.float32(wx[e, c])
            spcoefv[11, r] = np.float32(w1[e, c])
    return {
        "xb": np.ascontiguousarray(batch_b.astype(np.float32)),
        "stat": statv,
        "coef": coefv,
        "spfeat": vfeat,
        "spcoef": spcoefv,
    }


# ------------------------------------------------------------- bass program
def build_program(geom):
    import concourse.bacc as bacc
    import concourse.mybir as mybir
    import concourse.tile as tile

    CN, NTHn, THIRDn = geom["CN"], geom["NTH"], geom["THIRD"]
    rotate = geom["rotate"]
    FREEn = NTHn * THIRDn
    NSLn = THIRDn // 512
    nblk = CN * E * NSRC

    f32 = mybir.dt.float32
    bf16 = mybir.dt.bfloat16
    Alu = mybir.AluOpType
    Act = mybir.ActivationFunctionType

    nc = bacc.Bacc("TRN2", target_bir_lowering=False, debug=False, num_devices=B)
    xb = nc.dram_tensor("xb", [CN, P, FREEn], f32, kind="ExternalInput")
    stat = nc.dram_tensor("stat", [P, nblk * 32], f32, kind="ExternalInput")
    coef = nc.dram_tensor("coef", [P, CN * E], f32, kind="ExternalInput")
    spfeat = nc.dram_tensor("spfeat", [12, 256], f32, kind="ExternalInput")
    spcoef = nc.dram_tensor("spcoef", [12, E * C], f32, kind="ExternalInput")
    out = nc.dram_tensor("out", [E, CN, P, FREEn], f32, kind="ExternalOutput")
    spl = nc.dram_tensor("spl", [E * C, 256], f32, kind="ExternalOutput")

    with tile.TileContext(nc) as tc:
        with (
            tc.tile_pool(name="const", bufs=1) as constp,
            tc.tile_pool(name="xp", bufs=2) as xp,
            tc.tile_pool(name="feat", bufs=1) as featp,
            tc.tile_pool(name="yp", bufs=6) as yp,
            tc.tile_pool(name="psum", bufs=1, space="PSUM") as psp,
            tc.tile_pool(name="psum2", bufs=1, space="PSUM") as psp2,
        ):
            statt = constp.tile([P, nblk * 32], f32)
            nc.sync.dma_start(statt[:], stat.ap())
            # bf16 zero operands for the bank-opening zero-matmul
            zrow = constp.tile([1, 128], bf16)
            nc.vector.memset(zrow[:], 0.0)
            zin = constp.tile([1, 512], bf16)
            nc.vector.memset(zin[:], 0.0)
            coeft = constp.tile([P, CN * E], f32)
            nc.sync.dma_start(coeft[:], coef.ap())

            # ---- splines (tiny): one matmul [12,9]^T @ [12,256]
            spf = constp.tile([12, 256], f32)
            nc.sync.dma_start(spf[:], spfeat.ap())
            spc = constp.tile([12, E * C], f32)
            nc.sync.dma_start(spc[:], spcoef.ap())
            spps = psp2.tile([E * C, 256], f32)
            nc.tensor.matmul(spps[:], spc[:], spf[:], start=True, stop=True)
            spev = constp.tile([E * C, 256], f32)
            nc.vector.tensor_copy(spev[:], spps[:])
            nc.sync.dma_start(spl.ap(), spev[:])

            for _rep in range(geom.get("reps", 1)):
              for c in range(CN):
                for T in range(NTHn):
                    xt = xp.tile([P, THIRDn], f32, tag="x", name=f"x_{c}_{T}")
                    nc.sync.dma_start(
                        xt[:], xb.ap()[c, :, T * THIRDn:(T + 1) * THIRDn]
                    )
                    rsup = featp.tile([P, 9 * THIRDn], f32, tag="r", name=f"r_{c}_{T}")
                    ssup = featp.tile([P, 9 * THIRDn], f32, tag="s", name=f"s_{c}_{T}")
                    for m in range(9):
                        sl = slice(m * THIRDn, (m + 1) * THIRDn)
                        # r_m = relu(x - t_m)   (m=0 -> copy of x)
                        nc.vector.tensor_scalar(
                            rsup[:, sl], xt[:], -m * STEP, 0.0, Alu.add, Alu.max
                        )
                        # s_m = r_m^2
                        nc.scalar.activation(ssup[:, sl], rsup[:, sl], Act.Square)
                        # C_m = (x - t_m) * s_m   (overwrite r slot)
                        nc.vector.scalar_tensor_tensor(
                            rsup[:, sl], xt[:], -m * STEP, ssup[:, sl],
                            Alu.add, Alu.mult,
                        )
                    yst = [
                        yp.tile([P, THIRDn], f32, tag=f"yst{e}",
                                name=f"yst{e}_{c}_{T}", bufs=2)
                        for e in range(E)
                    ]
                    for s in range(NSLn):
                        ys_t = [
                            psp.tile([P, 512], f32, tag=f"Y{e}",
                                     name=f"Y{e}_{c}_{T}_{s}", bufs=2)
                            for e in range(E)
                        ]
                        for e in range(E):
                            # zero the bank + set has_written everywhere so
                            # the accumulating matmuls below can run in any
                            # order (maximal tile-position concurrency)
                            nc.tensor.matmul(
                                ys_t[e][:, :], zrow[:], zin[:],
                                start=True, stop=False, skip_group_check=True,
                            )
                        for src in range(NSRC):
                            if src < 9:
                                srcap = rsup[:, src * THIRDn + s * 512:
                                             src * THIRDn + s * 512 + 512]
                            elif src == 9:  # x^2
                                srcap = ssup[:, s * 512:s * 512 + 512]
                            else:           # x
                                srcap = xt[:, s * 512:s * 512 + 512]
                            for e in range(E):
                                nb = (c * E + e) * NSRC + src
                                lw = statt[:, nb * 32:(nb + 1) * 32]
                                for q in range(4):
                                    oq = (q + e) % 4 if rotate else q
                                    nc.tensor.matmul(
                                        ys_t[e][32 * oq:32 * oq + 32, :],
                                        lw[32 * q:32 * q + 32, :],
                                        srcap[32 * q:32 * q + 32, :],
                                        start=False,
                                        stop=(src == NSRC - 1),
                                        skip_group_check=True,
                                        tile_position=(32 * q, 32 * oq),
                                    )
                        for e in range(E):
                            nc.scalar.activation(
                                yst[e][:, s * 512:(s + 1) * 512],
                                ys_t[e][:], Act.Identity,
                                bias=coeft[:, c * E + e:c * E + e + 1],
                            )
                    # undo quadrant rotation: sbuf row 32*((q+e)%4)+p holds
                    # pixel 32*q+p
                    for e in range(E):
                        col = slice(T * THIRDn, (T + 1) * THIRDn)
                        k = (32 * e) % P if rotate else 0
                        if k == 0:
                            nc.sync.dma_start(out.ap()[e, c, :, col], yst[e][:])
                        else:
                            nc.sync.dma_start(
                                out.ap()[e, c, 0:P - k, col], yst[e][k:P, :]
                            )
                            nc.sync.dma_start(
                                out.ap()[e, c, P - k:P, col], yst[e][0:k, :]
                            )
    nc.compile()
    return nc


_PROG = None


def _get_prog():
    global _PROG
    if _PROG is None:
        _PROG = build_program(FULL_GEOM)
    return _PROG


# ------------------------------------------------------------------- driver
def kernel(batch, ys):
    """batch [8,3,768,768] f32, ys [8,3,3,10] f32 ->
    (out [8,3,3,768,768] f32, splines [8,3,3,255] f32)."""
    from concourse.bass_utils import run_bass_kernel_spmd

    batch = np.asarray(batch)
    ys = np.asarray(ys)
    in_maps = [
        host_inputs(batch[b].reshape(C, P, FREE), ys[b], FULL_GEOM)
        for b in range(B)
    ]
    nc = _get_prog()
    res = run_bass_kernel_spmd(nc, in_maps, list(range(B)))

    out = np.empty((B, E, C, H, W), np.float32)
    spl = np.empty((B, E, C, NVALS), np.float32)
    for b in range(B):
        out[b] = res.results[b]["out"].reshape(E, C, H, W)
        spl[b] = res.results[b]["spl"][:, :NVALS].reshape(E, C, NVALS)
    return out, spl
